# revision 35
# baseline (speedup 1.0000x reference)
"""Causal self-attention (B=2, T=2048, D_in=1152, D=1024, H=16) on 8 trn2 cores.

Sharding: 2-way data parallel over batch x 4-way tensor parallel over heads.
Core c handles batch b = c//4 and heads [4g, 4g+4) with g = c%4.

Per-core dataflow — hybrid precision so fp8 never touches short softmax rows:
  Query tile 0 (t < 512, small k_eff: errors don't average down) runs an
  all-bf16 pipeline; query tiles 1-3 (k_eff >= ~190) run fp8e4 with DoubleRow
  matmuls at 0.5 cyc/row. All weights are pre-scaled x32 on the host so fp8's
  e4m3 normal range covers their N(0,1/1152) entries; the resulting 1024x
  score scale folds into the exp scale, and onesm=1/32 folds the V prescale
  out during normalization. D_in is padded 1152->1280 = 5 DR chunk-pairs so
  fp8 projections never mix DR/non-DR in one psum accumulation group.
  QT = (32Wq)^T @ xp^T -> bf16 [128, 2, T] (head dims on partitions; cols
  0-511 from the bf16 proj, rest from fp8 DR proj); KT likewise. V stored as
  32v with a ones-column per head, in bf16 (blocks 0-3) and fp8 (all blocks).
  Scores transposed bf16: ST[k, q] = K Q^T per 128-row k-block; exp on ACT
  has bias -3 (cancels in softmax; keeps max exp ~e^4.6, under e4m3's 240
  max) and writes bf16 (qt0) or fp8 (qt>=1). Causal mask = one [128,2,256]
  mask-pair 0/1 multiply per diagonal group; block 2g+1's pre-diagonal
  stripe is zeroed so the paired DR att@V can't pick it up.
  OT~ = Vh~^T @ ex accumulates [65, 512] in PSUM — one DoubleRow matmul per
  two-block group for qt>=1 (the ex[:, jj, :] pair axis IS the DR layout),
  per-block bf16 matmuls for qt0; row 64 is the softmax row-sum. Normalize
  via reciprocal_approx_fast + rank-1 broadcast matmul, deferred into the
  filler stream. Y_partial = OT_all^T @ Wp in f32r as late filler.
Host sums the 4 partial Y per batch and adds bv@Wp + bp (exact: softmax
weights sum to 1, so the V bias contributes a constant row). Host
pre-shuffles all layouts so every DMA is 128 fully-contiguous lines.
"""

import functools as _ft
from collections import deque

import ml_dtypes
import numpy as np

import concourse.bass as bass
import concourse.mybir as mybir
import concourse.tile as tile
from concourse import bacc
from concourse.bass_utils import run_bass_kernel_spmd

F32 = mybir.dt.float32
F32R = mybir.dt.float32r
BF16 = mybir.dt.bfloat16
FP8 = mybir.dt.float8e4
AF = mybir.ActivationFunctionType
MUL = mybir.AluOpType.mult
DR = mybir.MatmulPerfMode.DoubleRow

B, T, DIN, D, H = 2, 2048, 1152, 1024, 16
HD = D // H           # 64 head dim
HLOC = 4              # heads per core
DLOC = HLOC * HD      # 256 local model dims
KC = DIN // 128       # 9 contraction chunks (bf16 path)
KCP = 10              # padded chunks (1280 = 5 DoubleRow pairs of 256)
NPAIR = KCP // 2
NT = T // 512         # 4 column tiles of 512
QC = T // 128         # 16 row chunks of 128
SCALE = 1.0 / np.sqrt(np.float32(HD))
WSC = 32.0            # host-side weight prescale for fp8 range
EBIAS = -3.0          # exp bias: cancels in softmax, keeps exp < e4m3 max

_CACHE = {}


def _build():
    nc = bacc.Bacc(None)

    xp8 = nc.dram_tensor("xp8", [128, 3, KCP, 512], FP8, kind="ExternalInput")
    xp16 = nc.dram_tensor("xp16", [128, KC, 512], BF16, kind="ExternalInput")
    wq8 = nc.dram_tensor("wq8", [128, KCP, DLOC], FP8, kind="ExternalInput")
    wk8 = nc.dram_tensor("wk8", [128, KCP, DLOC], FP8, kind="ExternalInput")
    wv8 = nc.dram_tensor("wv8", [128, KCP, DLOC], FP8, kind="ExternalInput")
    wq16 = nc.dram_tensor("wq16", [128, KC, DLOC], BF16, kind="ExternalInput")
    wk16 = nc.dram_tensor("wk16", [128, KC, DLOC], BF16, kind="ExternalInput")
    wv16 = nc.dram_tensor("wv16", [128, KC, DLOC], BF16, kind="ExternalInput")
    wp = nc.dram_tensor("wp", [128, 2, D], BF16, kind="ExternalInput")
    cbias = nc.dram_tensor("cbias", [128, 4], F32, kind="ExternalInput")
    c128 = nc.dram_tensor("c128", [128, 512], F32, kind="ExternalInput")
    conesm = nc.dram_tensor("conesm", [128, 256], BF16, kind="ExternalInput")
    y = nc.dram_tensor("y", [T, D], F32, kind="ExternalOutput")

    with tile.TileContext(nc) as tc:
        with (
            tc.tile_pool(name="const", bufs=1) as cpool,
            tc.tile_pool(name="work", bufs=2) as wpool,
            tc.tile_pool(name="exp", bufs=3) as epool,
            tc.tile_pool(name="stg", bufs=4) as spool,
            tc.tile_pool(name="psB", bufs=1, space="PSUM") as psB,
            tc.tile_pool(name="psC", bufs=2, space="PSUM") as psC,
            tc.tile_pool(name="psX", bufs=2, space="PSUM") as psX,
            nc.allow_low_precision(reason="fp8/bf16/f32r matmul pipeline"),
        ):
            t_wq8 = cpool.tile([128, KCP, DLOC], FP8, tag="t_wq8")
            t_wk8 = cpool.tile([128, KCP, DLOC], FP8, tag="t_wk8")
            t_wv8 = cpool.tile([128, KCP, DLOC], FP8, tag="t_wv8")
            t_wq16 = cpool.tile([128, KC, DLOC], BF16, tag="t_wq16")
            t_wk16 = cpool.tile([128, KC, DLOC], BF16, tag="t_wk16")
            t_wv16 = cpool.tile([128, KC, DLOC], BF16, tag="t_wv16")
            t_wp = cpool.tile([128, 2, D], BF16, tag="t_wp")
            t_cbias = cpool.tile([128, 4], F32, tag="t_cbias")
            t_c128 = cpool.tile([128, 512], F32R, tag="t_c128")
            t_conesm = cpool.tile([128, 256], BF16, tag="t_conesm")
            t_qt = cpool.tile([128, 2, T], BF16, tag="t_qt")
            t_kt = cpool.tile([128, 2, T], BF16, tag="t_kt")
            # HD+4: DoubleRow ldweights requires pair-axis stride %16 == 0
            # (HLOC*(HD+4) = 272 bytes); col 64 = softmax-sum ones, cols
            # 65-67 = dup ones (output rows 65-67 unused)
            t_v8 = cpool.tile([128, QC, HLOC, HD + 4], FP8, tag="t_v8")
            t_v16 = cpool.tile([128, 4, HLOC, HD + 1], BF16, tag="t_v16")
            t_ot = cpool.tile([128, 2, T], BF16, tag="t_ot")
            t_sums4 = cpool.tile([128, 512], F32, tag="t_sums4")
            t_recf = cpool.tile([128, 512], F32, tag="t_recf")
            t_rec4 = cpool.tile([128, 512], BF16, tag="t_rec4")

            t_bq = t_cbias[:, 0:2]
            t_bk = t_cbias[:, 2:4]
            t_onesm = t_conesm[:].rearrange("p (r c) -> p r c", r=4)
            t_tril = t_c128[:].rearrange("p (j c) -> p j c", j=2)

            # HAM warm-up: ~12 dummy matmuls on a memset tile run while the
            # first DMAs land, so the clock gate is at 8/8 when real matmuls
            # start (cold matmuls run at 1.2 GHz instead of 2.4)
            t_dmy = cpool.tile([128, 512], BF16, tag="t_dmy")
            nc.vector.memset(t_dmy[:], 0.5)
            p_dmy = psX.tile([128, 512], F32, tag="aux")
            for _ in range(10):
                nc.tensor.matmul(
                    p_dmy[:], t_dmy[:, 0:128], t_dmy[:], start=True, stop=True
                )

            # few LARGE DMAs: per-chunk loads are trigger-bound (~0.65us
            # per DMA_DIRECT2D issue + serialized ~1.3us/chunk transfers put
            # the last chunk ~18us out). Two halves per tensor match the
            # proj unit split while keeping trigger count low.
            t_xp0 = cpool.tile([128, KC, 512], BF16, tag="t_xp0")
            nc.sync.dma_start(t_wq16[:, 0 : KC // 2, :], wq16[:, 0 : KC // 2, :])
            nc.scalar.dma_start(t_xp0[:, 0 : KC // 2, :], xp16[:, 0 : KC // 2, :])
            nc.gpsimd.dma_start(t_wk16[:, 0 : KC // 2, :], wk16[:, 0 : KC // 2, :])
            nc.sync.dma_start(t_wq16[:, KC // 2 :, :], wq16[:, KC // 2 :, :])
            nc.scalar.dma_start(t_xp0[:, KC // 2 :, :], xp16[:, KC // 2 :, :])
            nc.gpsimd.dma_start(t_wk16[:, KC // 2 :, :], wk16[:, KC // 2 :, :])
            nc.sync.dma_start(t_wv16[:], wv16[:])
            nc.gpsimd.dma_start(t_cbias[:], cbias[:])
            nc.gpsimd.dma_start(t_c128[:], c128[:].bitcast(F32R))
            nc.gpsimd.dma_start(t_conesm[:], conesm[:])
            nc.gpsimd.dma_start(t_wq8[:], wq8[:])
            nc.gpsimd.dma_start(t_wk8[:], wk8[:])
            nc.gpsimd.dma_start(t_wv8[:], wv8[:])
            nc.gpsimd.memset(t_sums4[:], 1.0)
            # ones columns for every V block, written once through a staging
            # row (strided 1-byte memset is riskier than a strided copy)
            t_vones = cpool.tile([128, 256], F32, tag="t_vones")
            nc.gpsimd.memset(t_vones[:], 1.0)
            t_ebias = cpool.tile([128, 1], F32, tag="t_ebias")
            nc.gpsimd.memset(t_ebias[:], EBIAS)
            nc.vector.tensor_copy(
                out=t_v8[:, :, :, HD : HD + 4],
                in_=t_vones[:].rearrange("p (a b c) -> p a b c", a=QC, b=HLOC),
            )
            nc.vector.tensor_copy(
                out=t_v16[:, :, :, HD],
                in_=t_vones[:, 0:16].rearrange("p (a b) -> p a b", a=4),
            )

            def proj16():
                # bf16 projection for nt=0: accurate q/k cols 0-511 and
                # v blocks 0-3 (written to both the bf16 and fp8 V tiles)
                groups = []
                live = {}

                def qk_half(t_w, t_b, t_dst, m, half):
                    if half == 0:
                        p = psX.tile([128, 512], F32, tag="aux")
                        live[(id(t_w), m)] = p
                        ks = range(0, KC // 2)
                    else:
                        p = live.pop((id(t_w), m))
                        ks = range(KC // 2, KC)
                    for k in ks:
                        nc.tensor.matmul(
                            p[:],
                            t_w[:, k, 128 * m : 128 * m + 128],
                            t_xp0[:, k, :],
                            start=(k == 0),
                            stop=(k == KC - 1),
                        )
                    if half == 1:
                        # bias-add copy on DVE: the ACT queue is reserved for
                        # exp so proj epilogues never head-of-line block it
                        nc.vector.tensor_scalar(
                            t_dst[:, m, 0:512],
                            p[:],
                            t_b[:, m : m + 1],
                            None,
                            mybir.AluOpType.add,
                        )

                def v_half(tc4, half):
                    if half == 0:
                        p = psX.tile([128, 512], F32, tag="aux")
                        live[("v", tc4)] = p
                        ks = range(0, KC // 2)
                    else:
                        p = live.pop(("v", tc4))
                        ks = range(KC // 2, KC)
                    for k in ks:
                        nc.tensor.matmul(
                            p[:, :DLOC],
                            t_xp0[:, k, 128 * tc4 : 128 * tc4 + 128],
                            t_wv16[:, k, :],
                            start=(k == 0),
                            stop=(k == KC - 1),
                        )
                    if half == 1:
                        pv = p[:, :DLOC].rearrange("p (h d) -> p h d", h=HLOC)
                        nc.vector.tensor_copy(
                            out=t_v16[:, tc4, :, 0:HD], in_=pv
                        )
                        nc.vector.tensor_copy(
                            out=t_v8[:, tc4, :, 0:HD], in_=pv
                        )

                for t_w, t_b, t_dst in (
                    (t_wq16, t_bq, t_qt),
                    (t_wk16, t_bk, t_kt),
                ):
                    for m in range(2):
                        for half in range(2):
                            groups.append(
                                _ft.partial(qk_half, t_w, t_b, t_dst, m, half)
                            )
                for tc4 in range(4):
                    for half in range(2):
                        groups.append(_ft.partial(v_half, tc4, half))
                return groups

            def proj8(nt):
                # fp8 DoubleRow projection for nt>=1
                c0 = 512 * nt
                t_xp = wpool.tile([128, KCP, 512], FP8, tag="t_xp")
                nc.sync.dma_start(t_xp[:], xp8[:, nt - 1])
                groups = []
                live = {}

                def qk_half(t_w, t_b, t_dst, m, half, t_xp=t_xp, c0=c0):
                    if half == 0:
                        p = psX.tile([128, 512], F32, tag="aux")
                        live[(id(t_w), m)] = p
                        ps = range(0, 2)
                    else:
                        p = live.pop((id(t_w), m))
                        ps = range(2, NPAIR)
                    for pr in ps:
                        nc.tensor.matmul(
                            p[:],
                            t_w[:, 2 * pr : 2 * pr + 2, 128 * m : 128 * m + 128],
                            t_xp[:, 2 * pr : 2 * pr + 2, :],
                            start=(pr == 0),
                            stop=(pr == NPAIR - 1),
                            perf_mode=DR,
                        )
                    if half == 1:
                        nc.vector.tensor_scalar(
                            t_dst[:, m, c0 : c0 + 512],
                            p[:],
                            t_b[:, m : m + 1],
                            None,
                            mybir.AluOpType.add,
                        )

                def v_half(tc4, half, t_xp=t_xp, nt=nt):
                    tch = 4 * nt + tc4
                    if half == 0:
                        p = psX.tile([128, 512], F32, tag="aux")
                        live[("v", tc4)] = p
                        ps = range(0, 2)
                    else:
                        p = live.pop(("v", tc4))
                        ps = range(2, NPAIR)
                    for pr in ps:
                        nc.tensor.matmul(
                            p[:, :DLOC],
                            t_xp[
                                :, 2 * pr : 2 * pr + 2, 128 * tc4 : 128 * tc4 + 128
                            ],
                            t_wv8[:, 2 * pr : 2 * pr + 2, :],
                            start=(pr == 0),
                            stop=(pr == NPAIR - 1),
                            perf_mode=DR,
                        )
                    if half == 1:
                        nc.vector.tensor_copy(
                            out=t_v8[:, tch, :, 0:HD],
                            in_=p[:, :DLOC].rearrange("p (h d) -> p h d", h=HLOC),
                        )

                for t_w, t_b, t_dst in (
                    (t_wq8, t_bq, t_qt),
                    (t_wk8, t_bk, t_kt),
                ):
                    for m in range(2):
                        for half in range(2):
                            groups.append(
                                _ft.partial(qk_half, t_w, t_b, t_dst, m, half)
                            )
                for tc4 in range(4):
                    for half in range(2):
                        groups.append(_ft.partial(v_half, tc4, half))
                return groups

            pend = deque()       # proj groups: must drain on schedule
            pend_fqc = deque()   # output-proj chunks: reserved late filler

            # last-resort PE filler: a dependency-free dummy matmul keeps the
            # clock gate (HAM) at 8/8 — any PE gap over ~1us costs ~10us of
            # half-clock across every engine
            def dummy_mm():
                pd = psX.tile([128, 512], F32, tag="aux")
                nc.tensor.matmul(
                    pd[:], t_dmy[:, 0:128], t_dmy[:], start=True, stop=True
                )

            norm_emitted = set()

            def pop_filler(n, min_fqc=0, pad=False, fqc_ok=True):
                for _ in range(n):
                    if pend:
                        pend.popleft()()
                    elif (
                        fqc_ok
                        and len(pend_fqc) > min_fqc
                        # an O-proj unit is only valid once its query tile's
                        # normalization TTs have been emitted
                        and pend_fqc[0][0] in norm_emitted
                    ):
                        pend_fqc.popleft()[1]()
                    elif pad:
                        dummy_mm()

            # only what qt0-hf0 needs runs up front: Q-m0, K-m0 and V of
            # tile 0. The m=1 halves (first needed by qt0-hf1) and proj8(1..3)
            # become filler, so attention starts ~4us earlier.
            g16 = proj16()
            for i in (0, 1, 4, 5, *range(8, 16)):
                g16[i]()
            m1_left = [4]

            def _m1_unit(fn):
                def run():
                    fn()
                    m1_left[0] -= 1
                return run

            pend.extend(_m1_unit(g16[i]) for i in (2, 3, 6, 7))
            pend.extend(proj8(1))
            # wp issued here: lands during qt0, mostly off the proj window
            nc.gpsimd.dma_start(t_wp[:], wp[:])

            FLAGS = {"flush": False}

            def final_half(qc, n2, ty):
                # self-contained 512-col half of the output projection:
                # alloc->accumulate->stage within one filler unit so the aux
                # pool never rotates onto a live accumulation. N=512 keeps
                # the ldweights pipelined (a 256-col split costs ~2x).
                py = psX.tile([128, 512], F32, tag="aux")
                for c in range(2):
                    nc.tensor.matmul(
                        py[:],
                        t_ot[:, c, 128 * qc : 128 * qc + 128],
                        t_wp[:, c, 512 * n2 : 512 * n2 + 512],
                        start=(c == 0),
                        stop=(c == 1),
                    )
                # at flush alternate DVE/ACT so neither engine gates the drain
                if FLAGS["flush"] and n2 % 2 == 0:
                    nc.scalar.copy(ty[:, 512 * n2 : 512 * n2 + 512], py[:])
                else:
                    nc.vector.tensor_copy(
                        out=ty[:, 512 * n2 : 512 * n2 + 512], in_=py[:]
                    )
                if n2 == 1:
                    nc.gpsimd.dma_start(y[128 * qc : 128 * qc + 128, :], ty[:])

            def final_qc(qc, qt):
                # two poppable halves so late filler stays granular
                ty = wpool.tile([128, D], F32, tag="ty")
                for n2 in range(2):
                    pend_fqc.append(
                        (qt, _ft.partial(final_half, qc, n2, ty))
                    )

            def norm_qt(qt, stages):
                # deferred a full iteration, so 1/rowsum is long since ready
                q0 = 512 * qt
                for r in range(4):
                    hfr, pp = divmod(r, 2)
                    hp = 64 * pp
                    bcp = psX.tile([128, 512], F32, tag="aux")
                    nc.tensor.matmul(
                        bcp[0:64, :],
                        t_onesm[32 * r : 32 * r + 32, r, :],
                        t_rec4[32 * r : 32 * r + 32, :],
                        start=True,
                        stop=True,
                        tile_position=(32 * r, 0),
                    )
                    nc.vector.tensor_tensor(
                        t_ot[hp : hp + 64, hfr, q0 : q0 + 512],
                        bcp[0:64, :],
                        stages[r][:],
                        MUL,
                    )
                    if r == 1:
                        pop_filler(1)
                norm_emitted.add(qt)

            POPS = (4, 1, 1, 1)
            FQC_MIN = (0, 0, 0, 2)

            def make_stg(qt, hf, exs):
                q0 = 512 * qt
                fp8path = qt > 0

                def emit_stg(pp, g):
                    hp = 64 * pp
                    st = psB.tile([128, 2, 512], F32, tag=f"st{pp}")
                    # diagonal pairs skip columns below the pair's live
                    # region; block 2g+1's dead 128-col stripe above that
                    # is zeroed by the mask multiply (it must be exp'd
                    # anyway: the paired DR att@V reads both rows)
                    o = 256 * (g - 2 * qt) if g >= 2 * qt else 0
                    for jj in range(2):
                        j = 2 * g + jj
                        nc.tensor.matmul(
                            st[:, jj, o:],
                            t_kt[hp : hp + 64, hf, 128 * j : 128 * j + 128],
                            t_qt[hp : hp + 64, hf, q0 + o : q0 + 512],
                            start=True,
                            stop=True,
                        )
                    ex = epool.tile(
                        [128, 2, 512],
                        FP8 if fp8path else BF16,
                        tag=f"ex{pp}_{int(fp8path)}",
                    )
                    if g >= 2 * qt:
                        nc.scalar.activation(
                            ex[:, :, o:], st[:, :, o:], AF.Exp,
                            scale=float(SCALE / (WSC * WSC)),
                            bias=t_ebias[:],
                        )
                        eng = nc.vector if (pp == 0) else nc.gpsimd
                        eng.tensor_tensor(
                            ex[:, :, o : o + 256],
                            ex[:, :, o : o + 256],
                            t_tril,
                            MUL,
                        )
                    else:
                        nc.scalar.activation(
                            ex[:], st[:], AF.Exp,
                            scale=float(SCALE / (WSC * WSC)),
                            bias=t_ebias[:],
                        )
                    exs[(pp, g)] = ex

                return emit_stg

            sections = [(qt, hf) for qt in range(NT) for hf in range(2)]
            last_norm = None
            carry = None      # next section's pre-emitted exs dict
            stages = {}
            for si, (qt, hf) in enumerate(sections):
                # the pair's two heads run as independent, interleaved
                # ST->exp->OT chains: while one head's exp is on ACT, the
                # PE works the sibling head, so neither engine stalls.
                ngrp = 2 * qt + 2
                q0 = 512 * qt
                fp8path = qt > 0
                if hf == 0:
                    stages = {}
                    if qt + 2 < NT:
                        pend.extend(proj8(qt + 2))
                exs = carry if carry is not None else {}
                carry = None
                emit_stg = make_stg(qt, hf, exs)
                ots = {}

                def emit_otg(pp, g, first, last, qt=qt, hf=hf,
                             fp8path=fp8path, exs=exs, ots=ots):
                    h = 2 * hf + pp
                    ex = exs.pop((pp, g))
                    if fp8path:
                        o = 256 if g == 2 * qt + 1 else 0
                        nc.tensor.matmul(
                            ots[pp][:, o:],
                            t_v8[:, 2 * g : 2 * g + 2, h, :],
                            ex[:, :, o:],
                            start=first,
                            stop=last,
                            perf_mode=DR,
                        )
                    else:
                        for jj in range(2):
                            j = 2 * g + jj
                            off = 128 * j
                            nc.tensor.matmul(
                                ots[pp][0:65, off:],
                                t_v16[:, j, h, :],
                                ex[:, jj, off:],
                                start=(first and jj == 0),
                                stop=(last and jj == 1),
                            )

                if si == 0:
                    # first section: no previous section pre-emitted for us
                    emit_stg(0, 0)
                    emit_stg(1, 0)
                pop_filler(1, FQC_MIN[qt])
                if qt == 0 and hf == 1:
                    # hf1 scores read t_qt/t_kt m=1: those projection
                    # units must be emitted (not just queued) first
                    while m1_left[0] > 0:
                        pop_filler(1, fqc_ok=False)
                if hf == 0 and last_norm is not None:
                    # the qt-1 norm runs here: this section's first scores
                    # were pre-emitted by the previous section, so the PE has
                    # ready work while the rec4 DVE chain completes
                    last_norm()
                    last_norm = None
                for pp in range(2):
                    ots[pp] = psC.tile(
                        [68, 512], F32, tag="ot", name=f"ot_{qt}_{hf}_{pp}"
                    )
                for g in range(1, ngrp):
                    for pp in range(2):
                        emit_stg(pp, g)
                    for pp in range(2):
                        emit_otg(pp, g - 1, g == 1, False)
                    pop_filler(POPS[qt], FQC_MIN[qt], pad=True)
                pop_filler(2, FQC_MIN[qt], pad=True)
                for pp in range(2):
                    emit_otg(pp, ngrp - 1, ngrp == 1, True)
                # pre-emit the NEXT section's first score group before this
                # section's epilogue: the ACT queue stays fed across the
                # hf/qt boundary instead of idling ~1-2us
                if si + 1 < len(sections):
                    nqt, nhf = sections[si + 1]
                    if nqt == 0 and nhf == 1:
                        # the pre-emitted hf1 scores read t_qt/t_kt m=1:
                        # those projection units must be emitted first
                        while m1_left[0] > 0:
                            pop_filler(1, fqc_ok=False)
                    carry = {}
                    nstg = make_stg(nqt, nhf, carry)
                    nstg(0, 0)
                    nstg(1, 0)
                for pp in range(2):
                    h = 2 * hf + pp
                    # denominator row straight off PSUM so the reciprocal
                    # chain never waits on the big staging copies
                    nc.vector.tensor_copy(
                        out=t_sums4[32 * h : 32 * h + 1, :],
                        in_=ots[pp][64:65, :],
                    )
                    stage = spool.tile([64, 512], F32, tag="stg")
                    nc.vector.tensor_copy(
                        out=stage[:], in_=ots[pp][0:64, :]
                    )
                    stages[h] = stage
                if hf == 1:
                    # full-width fast reciprocal (custom-DVE ops misbehave
                    # on offset partition slices; per-lane cost is equal),
                    # rounded to f32r for the broadcast matmul
                    nc.vector.reciprocal_approx_fast(
                        out=t_recf[:], in_=t_sums4[:]
                    )
                    nc.vector.tensor_copy(out=t_rec4[:], in_=t_recf[:])
                    last_norm = _ft.partial(norm_qt, qt, stages)
                    for qc in range(4 * qt, 4 * qt + 4):
                        final_qc(qc, qt)

            pop_filler(4)
            last_norm()
            for _ in range(4):
                dummy_mm()
            FLAGS["flush"] = True
            while pend or pend_fqc:
                pop_filler(1)

    nc.compile()
    return nc


def _get_nc():
    if "nc" not in _CACHE:
        _CACHE["nc"] = _build()
    return _CACHE["nc"]


def _make_in_maps(xp, Wq, bq, Wk, bk, Wv, bv, Wp, bp):
    xp = np.asarray(xp, np.float32)
    Wq, Wk, Wv, Wp = (np.asarray(a, np.float32) for a in (Wq, Wk, Wv, Wp))
    bq, bk, bv, bp = (np.asarray(a, np.float32) for a in (bq, bk, bv, bp))
    FP8NP = ml_dtypes.float8_e4m3
    BF16NP = ml_dtypes.bfloat16

    tril = (np.arange(128)[None, :] >= np.arange(128)[:, None]).astype(
        np.float32
    )
    # mask-pair for a diagonal two-block group at column offset o:
    # block 2g (jj=0): cols 0:128 tril, 128:256 keep
    # block 2g+1 (jj=1): cols 0:128 dead (the DR pair reads them), 128:256 tril
    trilpair = np.empty((128, 2, 256), np.float32)
    trilpair[:, 0, 0:128] = tril
    trilpair[:, 0, 128:256] = 1.0
    trilpair[:, 1, 0:128] = 0.0
    trilpair[:, 1, 128:256] = tril
    # onesm = 1/WSC: folds the x32 V prescale out during normalization
    onesmv = np.zeros((128, 4, 64), np.float32)
    for r in range(4):
        onesmv[32 * r, r, :] = 1.0 / WSC
    c128v = trilpair.reshape(128, 512)
    conesmv = onesmv.reshape(128, 256).astype(BF16NP)

    def wshuf8(Wc):
        wpad = np.concatenate(
            [WSC * Wc, np.zeros((KCP * 128 - DIN, DLOC), np.float32)], axis=0
        )
        return np.ascontiguousarray(
            wpad.reshape(KCP, 128, DLOC).transpose(1, 0, 2)
        ).astype(FP8NP)

    def wshuf16(Wc):
        return np.ascontiguousarray(
            (WSC * Wc).reshape(KC, 128, DLOC).transpose(1, 0, 2)
        ).astype(BF16NP)

    in_maps = []
    for c in range(8):
        b, g = divmod(c, 4)
        s = slice(DLOC * g, DLOC * (g + 1))
        xpad = np.concatenate(
            [xp[b], np.zeros((T, KCP * 128 - DIN), np.float32)], axis=1
        )
        in_maps.append(
            {
                "xp8": np.ascontiguousarray(
                    xpad[512:].reshape(3, 512, KCP, 128).transpose(3, 0, 2, 1)
                ).astype(FP8NP),
                "xp16": np.ascontiguousarray(
                    xp[b, :512].reshape(512, KC, 128).transpose(2, 1, 0)
                ).astype(BF16NP),
                "wq8": wshuf8(Wq[:, s]),
                "wk8": wshuf8(Wk[:, s]),
                "wv8": wshuf8(Wv[:, s]),
                "wq16": wshuf16(Wq[:, s]),
                "wk16": wshuf16(Wk[:, s]),
                "wv16": wshuf16(Wv[:, s]),
                "wp": np.ascontiguousarray(
                    Wp[s, :].reshape(2, 128, D).transpose(1, 0, 2)
                ).astype(BF16NP),
                "cbias": np.ascontiguousarray(
                    WSC
                    * np.concatenate(
                        [bq[s].reshape(2, 128).T, bk[s].reshape(2, 128).T], 1
                    )
                ),
                "c128": c128v,
                "conesm": conesmv,
            }
        )

    return in_maps


def _gather(results, bv, Wp, bp):
    out = np.zeros((B, T, D), np.float32)
    for c in range(8):
        out[c // 4] += results[c]["y"]
    # softmax weights sum to exactly 1, so the V bias contributes the
    # constant row bv @ Wp — applied here instead of on-device
    const = np.asarray(bv, np.float32) @ np.asarray(Wp, np.float32)
    out += (const + np.asarray(bp, np.float32))[None, None, :]
    return out


def kernel(xp, Wq, bq, Wk, bk, Wv, bv, Wp, bp):
    nc = _get_nc()
    in_maps = _make_in_maps(xp, Wq, bq, Wk, bk, Wv, bv, Wp, bp)
    res = run_bass_kernel_spmd(nc, in_maps, list(range(8)))
    return _gather(res.results, bv, Wp, bp)


# revision 36
# speedup vs baseline: 1.0312x; 1.0312x over previous
"""Causal self-attention (B=2, T=2048, D_in=1152, D=1024, H=16) on 8 trn2 cores.

Sharding: 2-way data parallel over batch x 4-way tensor parallel over heads.
Core c handles batch b = c//4 and heads [4g, 4g+4) with g = c%4.

Per-core dataflow — hybrid precision so fp8 never touches short softmax rows:
  Query tile 0 (t < 512, small k_eff: errors don't average down) runs an
  all-bf16 pipeline; query tiles 1-3 (k_eff >= ~190) run fp8e4 with DoubleRow
  matmuls at 0.5 cyc/row. All weights are pre-scaled x32 on the host so fp8's
  e4m3 normal range covers their N(0,1/1152) entries; the resulting 1024x
  score scale folds into the exp scale, and onesm=1/32 folds the V prescale
  out during normalization. D_in is padded 1152->1280 = 5 DR chunk-pairs so
  fp8 projections never mix DR/non-DR in one psum accumulation group.
  QT = (32Wq)^T @ xp^T -> bf16 [128, 2, T] (head dims on partitions; cols
  0-511 from the bf16 proj, rest from fp8 DR proj); KT likewise. V stored as
  32v with a ones-column per head, in bf16 (blocks 0-3) and fp8 (all blocks).
  Scores transposed bf16: ST[k, q] = K Q^T per 128-row k-block; exp on ACT
  has bias -3 (cancels in softmax; keeps max exp ~e^4.6, under e4m3's 240
  max) and writes bf16 (qt0) or fp8 (qt>=1). Causal mask = one [128,2,256]
  mask-pair 0/1 multiply per diagonal group; block 2g+1's pre-diagonal
  stripe is zeroed so the paired DR att@V can't pick it up.
  OT~ = Vh~^T @ ex accumulates [65, 512] in PSUM — one DoubleRow matmul per
  two-block group for qt>=1 (the ex[:, jj, :] pair axis IS the DR layout),
  per-block bf16 matmuls for qt0; row 64 is the softmax row-sum. Normalize
  via reciprocal_approx_fast + rank-1 broadcast matmul, deferred into the
  filler stream. Y_partial = OT_all^T @ Wp in f32r as late filler.
Host sums the 4 partial Y per batch and adds bv@Wp + bp (exact: softmax
weights sum to 1, so the V bias contributes a constant row). Host
pre-shuffles all layouts so every DMA is 128 fully-contiguous lines.
"""

import functools as _ft
from collections import deque

import ml_dtypes
import numpy as np

import concourse.bass as bass
import concourse.mybir as mybir
import concourse.tile as tile
from concourse import bacc
from concourse.bass_utils import run_bass_kernel_spmd

F32 = mybir.dt.float32
F32R = mybir.dt.float32r
BF16 = mybir.dt.bfloat16
FP8 = mybir.dt.float8e4
AF = mybir.ActivationFunctionType
MUL = mybir.AluOpType.mult
DR = mybir.MatmulPerfMode.DoubleRow

B, T, DIN, D, H = 2, 2048, 1152, 1024, 16
HD = D // H           # 64 head dim
HLOC = 4              # heads per core
DLOC = HLOC * HD      # 256 local model dims
KC = DIN // 128       # 9 contraction chunks (bf16 path)
KCP = 10              # padded chunks (1280 = 5 DoubleRow pairs of 256)
NPAIR = KCP // 2
NT = T // 512         # 4 column tiles of 512
QC = T // 128         # 16 row chunks of 128
SCALE = 1.0 / np.sqrt(np.float32(HD))
WSC = 32.0            # host-side weight prescale for fp8 range
EBIAS = -3.0          # exp bias: cancels in softmax, keeps exp < e4m3 max

_CACHE = {}


def _build():
    nc = bacc.Bacc(None)

    xp8 = nc.dram_tensor("xp8", [128, 3, KCP, 512], FP8, kind="ExternalInput")
    xp16 = nc.dram_tensor("xp16", [128, KC, 512], BF16, kind="ExternalInput")
    wq8 = nc.dram_tensor("wq8", [128, KCP, DLOC], FP8, kind="ExternalInput")
    wk8 = nc.dram_tensor("wk8", [128, KCP, DLOC], FP8, kind="ExternalInput")
    wv8 = nc.dram_tensor("wv8", [128, KCP, DLOC], FP8, kind="ExternalInput")
    wq16 = nc.dram_tensor("wq16", [128, KC, DLOC], BF16, kind="ExternalInput")
    wk16 = nc.dram_tensor("wk16", [128, KC, DLOC], BF16, kind="ExternalInput")
    wv16 = nc.dram_tensor("wv16", [128, KC, DLOC], BF16, kind="ExternalInput")
    wp = nc.dram_tensor("wp", [128, 2, D], BF16, kind="ExternalInput")
    cbias = nc.dram_tensor("cbias", [128, 4], F32, kind="ExternalInput")
    c128 = nc.dram_tensor("c128", [128, 512], F32, kind="ExternalInput")
    conesm = nc.dram_tensor("conesm", [128, 256], BF16, kind="ExternalInput")
    y = nc.dram_tensor("y", [T, D], F32, kind="ExternalOutput")

    with tile.TileContext(nc) as tc:
        with (
            tc.tile_pool(name="const", bufs=1) as cpool,
            tc.tile_pool(name="work", bufs=2) as wpool,
            tc.tile_pool(name="exp", bufs=3) as epool,
            tc.tile_pool(name="stg", bufs=4) as spool,
            tc.tile_pool(name="psB", bufs=1, space="PSUM") as psB,
            tc.tile_pool(name="psC", bufs=2, space="PSUM") as psC,
            tc.tile_pool(name="psX", bufs=2, space="PSUM") as psX,
            nc.allow_low_precision(reason="fp8/bf16/f32r matmul pipeline"),
        ):
            t_wq8 = cpool.tile([128, KCP, DLOC], FP8, tag="t_wq8")
            t_wk8 = cpool.tile([128, KCP, DLOC], FP8, tag="t_wk8")
            t_wv8 = cpool.tile([128, KCP, DLOC], FP8, tag="t_wv8")
            t_wq16 = cpool.tile([128, KC, DLOC], BF16, tag="t_wq16")
            t_wk16 = cpool.tile([128, KC, DLOC], BF16, tag="t_wk16")
            t_wv16 = cpool.tile([128, KC, DLOC], BF16, tag="t_wv16")
            t_wp = cpool.tile([128, 2, D], BF16, tag="t_wp")
            t_cbias = cpool.tile([128, 4], F32, tag="t_cbias")
            t_c128 = cpool.tile([128, 512], F32R, tag="t_c128")
            t_conesm = cpool.tile([128, 256], BF16, tag="t_conesm")
            t_qt = cpool.tile([128, 2, T], BF16, tag="t_qt")
            t_kt = cpool.tile([128, 2, T], BF16, tag="t_kt")
            # HD+4: DoubleRow ldweights requires pair-axis stride %16 == 0
            # (HLOC*(HD+4) = 272 bytes); col 64 = softmax-sum ones, cols
            # 65-67 = dup ones (output rows 65-67 unused)
            t_v8 = cpool.tile([128, QC, HLOC, HD + 4], FP8, tag="t_v8")
            t_v16 = cpool.tile([128, 4, HLOC, HD + 1], BF16, tag="t_v16")
            t_ot = cpool.tile([128, 2, T], BF16, tag="t_ot")
            t_sums4 = cpool.tile([128, 512], F32, tag="t_sums4")
            t_recf = cpool.tile([128, 512], F32, tag="t_recf")
            t_rec4 = cpool.tile([128, 512], BF16, tag="t_rec4")

            t_bq = t_cbias[:, 0:2]
            t_bk = t_cbias[:, 2:4]
            t_onesm = t_conesm[:].rearrange("p (r c) -> p r c", r=4)
            t_tril = t_c128[:].rearrange("p (j c) -> p j c", j=2)

            # HAM warm-up: ~12 dummy matmuls on a memset tile run while the
            # first DMAs land, so the clock gate is at 8/8 when real matmuls
            # start (cold matmuls run at 1.2 GHz instead of 2.4)
            t_dmy = cpool.tile([128, 512], BF16, tag="t_dmy")
            nc.vector.memset(t_dmy[:], 0.5)
            p_dmy = psX.tile([128, 512], F32, tag="aux")
            for _ in range(10):
                nc.tensor.matmul(
                    p_dmy[:], t_dmy[:, 0:128], t_dmy[:], start=True, stop=True
                )

            # few LARGE DMAs: per-chunk loads are trigger-bound (~0.65us
            # per DMA_DIRECT2D issue + serialized ~1.3us/chunk transfers put
            # the last chunk ~18us out). Two halves per tensor match the
            # proj unit split while keeping trigger count low.
            t_xp0 = cpool.tile([128, KC, 512], BF16, tag="t_xp0")
            nc.sync.dma_start(t_wq16[:, 0 : KC // 2, :], wq16[:, 0 : KC // 2, :])
            nc.scalar.dma_start(t_xp0[:, 0 : KC // 2, :], xp16[:, 0 : KC // 2, :])
            nc.gpsimd.dma_start(t_wk16[:, 0 : KC // 2, :], wk16[:, 0 : KC // 2, :])
            nc.sync.dma_start(t_wq16[:, KC // 2 :, :], wq16[:, KC // 2 :, :])
            nc.scalar.dma_start(t_xp0[:, KC // 2 :, :], xp16[:, KC // 2 :, :])
            nc.gpsimd.dma_start(t_wk16[:, KC // 2 :, :], wk16[:, KC // 2 :, :])
            nc.sync.dma_start(t_wv16[:], wv16[:])
            nc.gpsimd.dma_start(t_cbias[:], cbias[:])
            nc.gpsimd.dma_start(t_c128[:], c128[:].bitcast(F32R))
            nc.gpsimd.dma_start(t_conesm[:], conesm[:])
            nc.gpsimd.dma_start(t_wq8[:], wq8[:])
            nc.gpsimd.dma_start(t_wk8[:], wk8[:])
            nc.gpsimd.dma_start(t_wv8[:], wv8[:])
            nc.gpsimd.memset(t_sums4[:], 1.0)
            # ones columns for every V block, written once through a staging
            # row (strided 1-byte memset is riskier than a strided copy)
            t_vones = cpool.tile([128, 256], F32, tag="t_vones")
            nc.gpsimd.memset(t_vones[:], 1.0)
            t_ebias = cpool.tile([128, 1], F32, tag="t_ebias")
            nc.gpsimd.memset(t_ebias[:], EBIAS)
            nc.vector.tensor_copy(
                out=t_v8[:, :, :, HD : HD + 4],
                in_=t_vones[:].rearrange("p (a b c) -> p a b c", a=QC, b=HLOC),
            )
            nc.vector.tensor_copy(
                out=t_v16[:, :, :, HD],
                in_=t_vones[:, 0:16].rearrange("p (a b) -> p a b", a=4),
            )

            def proj16():
                # bf16 projection for nt=0: accurate q/k cols 0-511 and
                # v blocks 0-3 (written to both the bf16 and fp8 V tiles)
                groups = []
                live = {}

                def qk_half(t_w, t_b, t_dst, m, half):
                    if half == 0:
                        p = psX.tile([128, 512], F32, tag="aux")
                        live[(id(t_w), m)] = p
                        ks = range(0, KC // 2)
                    else:
                        p = live.pop((id(t_w), m))
                        ks = range(KC // 2, KC)
                    for k in ks:
                        nc.tensor.matmul(
                            p[:],
                            t_w[:, k, 128 * m : 128 * m + 128],
                            t_xp0[:, k, :],
                            start=(k == 0),
                            stop=(k == KC - 1),
                        )
                    if half == 1:
                        # bias-add copy on DVE: the ACT queue is reserved for
                        # exp so proj epilogues never head-of-line block it
                        nc.vector.tensor_scalar(
                            t_dst[:, m, 0:512],
                            p[:],
                            t_b[:, m : m + 1],
                            None,
                            mybir.AluOpType.add,
                        )

                def v_half(tc4, half):
                    if half == 0:
                        p = psX.tile([128, 512], F32, tag="aux")
                        live[("v", tc4)] = p
                        ks = range(0, KC // 2)
                    else:
                        p = live.pop(("v", tc4))
                        ks = range(KC // 2, KC)
                    for k in ks:
                        nc.tensor.matmul(
                            p[:, :DLOC],
                            t_xp0[:, k, 128 * tc4 : 128 * tc4 + 128],
                            t_wv16[:, k, :],
                            start=(k == 0),
                            stop=(k == KC - 1),
                        )
                    if half == 1:
                        pv = p[:, :DLOC].rearrange("p (h d) -> p h d", h=HLOC)
                        nc.vector.tensor_copy(
                            out=t_v16[:, tc4, :, 0:HD], in_=pv
                        )
                        nc.vector.tensor_copy(
                            out=t_v8[:, tc4, :, 0:HD], in_=pv
                        )

                for t_w, t_b, t_dst in (
                    (t_wq16, t_bq, t_qt),
                    (t_wk16, t_bk, t_kt),
                ):
                    for m in range(2):
                        for half in range(2):
                            groups.append(
                                _ft.partial(qk_half, t_w, t_b, t_dst, m, half)
                            )
                for tc4 in range(4):
                    for half in range(2):
                        groups.append(_ft.partial(v_half, tc4, half))
                return groups

            def proj8(nt):
                # fp8 DoubleRow projection for nt>=1
                c0 = 512 * nt
                t_xp = wpool.tile([128, KCP, 512], FP8, tag="t_xp")
                nc.sync.dma_start(t_xp[:], xp8[:, nt - 1])
                groups = []
                live = {}

                def qk_half(t_w, t_b, t_dst, m, half, t_xp=t_xp, c0=c0):
                    if half == 0:
                        p = psX.tile([128, 512], F32, tag="aux")
                        live[(id(t_w), m)] = p
                        ps = range(0, 2)
                    else:
                        p = live.pop((id(t_w), m))
                        ps = range(2, NPAIR)
                    for pr in ps:
                        nc.tensor.matmul(
                            p[:],
                            t_w[:, 2 * pr : 2 * pr + 2, 128 * m : 128 * m + 128],
                            t_xp[:, 2 * pr : 2 * pr + 2, :],
                            start=(pr == 0),
                            stop=(pr == NPAIR - 1),
                            perf_mode=DR,
                        )
                    if half == 1:
                        nc.vector.tensor_scalar(
                            t_dst[:, m, c0 : c0 + 512],
                            p[:],
                            t_b[:, m : m + 1],
                            None,
                            mybir.AluOpType.add,
                        )

                def v_half(tc4, half, t_xp=t_xp, nt=nt):
                    tch = 4 * nt + tc4
                    if half == 0:
                        p = psX.tile([128, 512], F32, tag="aux")
                        live[("v", tc4)] = p
                        ps = range(0, 2)
                    else:
                        p = live.pop(("v", tc4))
                        ps = range(2, NPAIR)
                    for pr in ps:
                        nc.tensor.matmul(
                            p[:, :DLOC],
                            t_xp[
                                :, 2 * pr : 2 * pr + 2, 128 * tc4 : 128 * tc4 + 128
                            ],
                            t_wv8[:, 2 * pr : 2 * pr + 2, :],
                            start=(pr == 0),
                            stop=(pr == NPAIR - 1),
                            perf_mode=DR,
                        )
                    if half == 1:
                        nc.vector.tensor_copy(
                            out=t_v8[:, tch, :, 0:HD],
                            in_=p[:, :DLOC].rearrange("p (h d) -> p h d", h=HLOC),
                        )

                for t_w, t_b, t_dst in (
                    (t_wq8, t_bq, t_qt),
                    (t_wk8, t_bk, t_kt),
                ):
                    for m in range(2):
                        for half in range(2):
                            groups.append(
                                _ft.partial(qk_half, t_w, t_b, t_dst, m, half)
                            )
                for tc4 in range(4):
                    for half in range(2):
                        groups.append(_ft.partial(v_half, tc4, half))
                return groups

            pend = deque()       # proj groups: must drain on schedule
            pend_fqc = deque()   # output-proj chunks: reserved late filler

            # last-resort PE filler: a dependency-free dummy matmul keeps the
            # clock gate (HAM) at 8/8 — any PE gap over ~1us costs ~10us of
            # half-clock across every engine
            def dummy_mm():
                pd = psX.tile([128, 512], F32, tag="aux")
                nc.tensor.matmul(
                    pd[:], t_dmy[:, 0:128], t_dmy[:], start=True, stop=True
                )

            norm_emitted = set()

            def pop_filler(n, min_fqc=0, pad=False, fqc_ok=True):
                for _ in range(n):
                    if pend:
                        pend.popleft()()
                    elif (
                        fqc_ok
                        and len(pend_fqc) > min_fqc
                        # an O-proj unit is only valid once its query tile's
                        # normalization TTs have been emitted
                        and pend_fqc[0][0] in norm_emitted
                    ):
                        pend_fqc.popleft()[1]()
                    elif pad:
                        dummy_mm()

            # only what qt0-hf0 needs runs up front: Q-m0, K-m0 and V of
            # tile 0. The m=1 halves (first needed by qt0-hf1) and proj8(1..3)
            # become filler, so attention starts ~4us earlier.
            g16 = proj16()
            for i in (0, 1, 4, 5, *range(8, 16)):
                g16[i]()
            m1_left = [4]

            def _m1_unit(fn):
                def run():
                    fn()
                    m1_left[0] -= 1
                return run

            pend.extend(_m1_unit(g16[i]) for i in (2, 3, 6, 7))
            pend.extend(proj8(1))
            # wp issued here: lands during qt0, mostly off the proj window
            nc.gpsimd.dma_start(t_wp[:], wp[:])

            FLAGS = {"flush": False}

            def final_half(qc, n2, ty):
                # self-contained 512-col half of the output projection:
                # alloc->accumulate->stage within one filler unit so the aux
                # pool never rotates onto a live accumulation. N=512 keeps
                # the ldweights pipelined (a 256-col split costs ~2x).
                py = psX.tile([128, 512], F32, tag="aux")
                for c in range(2):
                    nc.tensor.matmul(
                        py[:],
                        t_ot[:, c, 128 * qc : 128 * qc + 128],
                        t_wp[:, c, 512 * n2 : 512 * n2 + 512],
                        start=(c == 0),
                        stop=(c == 1),
                    )
                # at flush alternate DVE/ACT so neither engine gates the drain
                if FLAGS["flush"] and n2 % 2 == 0:
                    nc.scalar.copy(ty[:, 512 * n2 : 512 * n2 + 512], py[:])
                else:
                    nc.vector.tensor_copy(
                        out=ty[:, 512 * n2 : 512 * n2 + 512], in_=py[:]
                    )
                if n2 == 1:
                    nc.gpsimd.dma_start(y[128 * qc : 128 * qc + 128, :], ty[:])

            def final_qc(qc, qt):
                # two poppable halves so late filler stays granular
                ty = wpool.tile([128, D], F32, tag="ty")
                for n2 in range(2):
                    pend_fqc.append(
                        (qt, _ft.partial(final_half, qc, n2, ty))
                    )

            def norm_qt(qt, stages):
                # deferred a full iteration, so 1/rowsum is long since ready
                q0 = 512 * qt
                for r in range(4):
                    hfr, pp = divmod(r, 2)
                    hp = 64 * pp
                    bcp = psX.tile([128, 512], F32, tag="aux")
                    nc.tensor.matmul(
                        bcp[0:64, :],
                        t_onesm[32 * r : 32 * r + 32, r, :],
                        t_rec4[32 * r : 32 * r + 32, :],
                        start=True,
                        stop=True,
                        tile_position=(32 * r, 0),
                    )
                    nc.vector.tensor_tensor(
                        t_ot[hp : hp + 64, hfr, q0 : q0 + 512],
                        bcp[0:64, :],
                        stages[r][:],
                        MUL,
                    )
                    if r == 1:
                        pop_filler(1)
                norm_emitted.add(qt)

            POPS = (4, 2, 2, 1)
            FQC_MIN = (0, 0, 0, 2)

            def make_stg(qt, hf, exs):
                q0 = 512 * qt
                fp8path = qt > 0

                def emit_stg(pp, g):
                    hp = 64 * pp
                    st = psB.tile([128, 2, 512], F32, tag=f"st{pp}")
                    # diagonal pairs skip columns below the pair's live
                    # region; block 2g+1's dead 128-col stripe above that
                    # is zeroed by the mask multiply (it must be exp'd
                    # anyway: the paired DR att@V reads both rows)
                    o = 256 * (g - 2 * qt) if g >= 2 * qt else 0
                    for jj in range(2):
                        j = 2 * g + jj
                        nc.tensor.matmul(
                            st[:, jj, o:],
                            t_kt[hp : hp + 64, hf, 128 * j : 128 * j + 128],
                            t_qt[hp : hp + 64, hf, q0 + o : q0 + 512],
                            start=True,
                            stop=True,
                        )
                    ex = epool.tile(
                        [128, 2, 512],
                        FP8 if fp8path else BF16,
                        tag=f"ex{pp}_{int(fp8path)}",
                    )
                    if g >= 2 * qt:
                        nc.scalar.activation(
                            ex[:, :, o:], st[:, :, o:], AF.Exp,
                            scale=float(SCALE / (WSC * WSC)),
                            bias=t_ebias[:],
                        )
                        eng = nc.vector if (pp == 0) else nc.gpsimd
                        eng.tensor_tensor(
                            ex[:, :, o : o + 256],
                            ex[:, :, o : o + 256],
                            t_tril,
                            MUL,
                        )
                    else:
                        nc.scalar.activation(
                            ex[:], st[:], AF.Exp,
                            scale=float(SCALE / (WSC * WSC)),
                            bias=t_ebias[:],
                        )
                    exs[(pp, g)] = ex

                return emit_stg

            sections = [(qt, hf) for qt in range(NT) for hf in range(2)]
            last_norm = None
            carry = None      # next section's pre-emitted exs dict
            stages = {}
            for si, (qt, hf) in enumerate(sections):
                # the pair's two heads run as independent, interleaved
                # ST->exp->OT chains: while one head's exp is on ACT, the
                # PE works the sibling head, so neither engine stalls.
                ngrp = 2 * qt + 2
                q0 = 512 * qt
                fp8path = qt > 0
                if hf == 0:
                    stages = {}
                    if qt + 2 < NT:
                        pend.extend(proj8(qt + 2))
                exs = carry if carry is not None else {}
                carry = None
                emit_stg = make_stg(qt, hf, exs)
                ots = {}

                def emit_otg(pp, g, first, last, qt=qt, hf=hf,
                             fp8path=fp8path, exs=exs, ots=ots):
                    h = 2 * hf + pp
                    ex = exs.pop((pp, g))
                    if fp8path:
                        o = 256 if g == 2 * qt + 1 else 0
                        nc.tensor.matmul(
                            ots[pp][:, o:],
                            t_v8[:, 2 * g : 2 * g + 2, h, :],
                            ex[:, :, o:],
                            start=first,
                            stop=last,
                            perf_mode=DR,
                        )
                    else:
                        for jj in range(2):
                            j = 2 * g + jj
                            off = 128 * j
                            nc.tensor.matmul(
                                ots[pp][0:65, off:],
                                t_v16[:, j, h, :],
                                ex[:, jj, off:],
                                start=(first and jj == 0),
                                stop=(last and jj == 1),
                            )

                if si == 0:
                    # first section: no previous section pre-emitted for us
                    emit_stg(0, 0)
                    emit_stg(1, 0)
                pop_filler(1, FQC_MIN[qt])
                if qt == 0 and hf == 1:
                    # hf1 scores read t_qt/t_kt m=1: those projection
                    # units must be emitted (not just queued) first
                    while m1_left[0] > 0:
                        pop_filler(1, fqc_ok=False)
                if hf == 0 and last_norm is not None:
                    # the qt-1 norm runs here: this section's first scores
                    # were pre-emitted by the previous section, so the PE has
                    # ready work while the rec4 DVE chain completes
                    last_norm()
                    last_norm = None
                for pp in range(2):
                    ots[pp] = psC.tile(
                        [68, 512], F32, tag="ot", name=f"ot_{qt}_{hf}_{pp}"
                    )
                for g in range(1, ngrp):
                    for pp in range(2):
                        emit_stg(pp, g)
                    for pp in range(2):
                        emit_otg(pp, g - 1, g == 1, False)
                    pop_filler(POPS[qt], FQC_MIN[qt], pad=True)
                pop_filler(2, FQC_MIN[qt], pad=True)
                for pp in range(2):
                    emit_otg(pp, ngrp - 1, ngrp == 1, True)
                # pre-emit the NEXT section's first score group before this
                # section's epilogue: the ACT queue stays fed across the
                # hf/qt boundary instead of idling ~1-2us
                if si + 1 < len(sections):
                    nqt, nhf = sections[si + 1]
                    if nqt == 0 and nhf == 1:
                        # the pre-emitted hf1 scores read t_qt/t_kt m=1:
                        # those projection units must be emitted first
                        while m1_left[0] > 0:
                            pop_filler(1, fqc_ok=False)
                    carry = {}
                    nstg = make_stg(nqt, nhf, carry)
                    nstg(0, 0)
                    nstg(1, 0)
                for pp in range(2):
                    h = 2 * hf + pp
                    # denominator row straight off PSUM so the reciprocal
                    # chain never waits on the big staging copies
                    nc.vector.tensor_copy(
                        out=t_sums4[32 * h : 32 * h + 1, :],
                        in_=ots[pp][64:65, :],
                    )
                    stage = spool.tile([64, 512], F32, tag="stg")
                    nc.vector.tensor_copy(
                        out=stage[:], in_=ots[pp][0:64, :]
                    )
                    stages[h] = stage
                if hf == 1:
                    # full-width fast reciprocal (custom-DVE ops misbehave
                    # on offset partition slices; per-lane cost is equal),
                    # rounded to f32r for the broadcast matmul
                    nc.vector.reciprocal_approx_fast(
                        out=t_recf[:], in_=t_sums4[:]
                    )
                    nc.vector.tensor_copy(out=t_rec4[:], in_=t_recf[:])
                    last_norm = _ft.partial(norm_qt, qt, stages)
                    for qc in range(4 * qt, 4 * qt + 4):
                        final_qc(qc, qt)

            pop_filler(4)
            last_norm()
            for _ in range(4):
                dummy_mm()
            FLAGS["flush"] = True
            while pend or pend_fqc:
                pop_filler(1)

    nc.compile()
    return nc


def _get_nc():
    if "nc" not in _CACHE:
        _CACHE["nc"] = _build()
    return _CACHE["nc"]


def _make_in_maps(xp, Wq, bq, Wk, bk, Wv, bv, Wp, bp):
    xp = np.asarray(xp, np.float32)
    Wq, Wk, Wv, Wp = (np.asarray(a, np.float32) for a in (Wq, Wk, Wv, Wp))
    bq, bk, bv, bp = (np.asarray(a, np.float32) for a in (bq, bk, bv, bp))
    FP8NP = ml_dtypes.float8_e4m3
    BF16NP = ml_dtypes.bfloat16

    tril = (np.arange(128)[None, :] >= np.arange(128)[:, None]).astype(
        np.float32
    )
    # mask-pair for a diagonal two-block group at column offset o:
    # block 2g (jj=0): cols 0:128 tril, 128:256 keep
    # block 2g+1 (jj=1): cols 0:128 dead (the DR pair reads them), 128:256 tril
    trilpair = np.empty((128, 2, 256), np.float32)
    trilpair[:, 0, 0:128] = tril
    trilpair[:, 0, 128:256] = 1.0
    trilpair[:, 1, 0:128] = 0.0
    trilpair[:, 1, 128:256] = tril
    # onesm = 1/WSC: folds the x32 V prescale out during normalization
    onesmv = np.zeros((128, 4, 64), np.float32)
    for r in range(4):
        onesmv[32 * r, r, :] = 1.0 / WSC
    c128v = trilpair.reshape(128, 512)
    conesmv = onesmv.reshape(128, 256).astype(BF16NP)

    def wshuf8(Wc):
        wpad = np.concatenate(
            [WSC * Wc, np.zeros((KCP * 128 - DIN, DLOC), np.float32)], axis=0
        )
        return np.ascontiguousarray(
            wpad.reshape(KCP, 128, DLOC).transpose(1, 0, 2)
        ).astype(FP8NP)

    def wshuf16(Wc):
        return np.ascontiguousarray(
            (WSC * Wc).reshape(KC, 128, DLOC).transpose(1, 0, 2)
        ).astype(BF16NP)

    in_maps = []
    for c in range(8):
        b, g = divmod(c, 4)
        s = slice(DLOC * g, DLOC * (g + 1))
        xpad = np.concatenate(
            [xp[b], np.zeros((T, KCP * 128 - DIN), np.float32)], axis=1
        )
        in_maps.append(
            {
                "xp8": np.ascontiguousarray(
                    xpad[512:].reshape(3, 512, KCP, 128).transpose(3, 0, 2, 1)
                ).astype(FP8NP),
                "xp16": np.ascontiguousarray(
                    xp[b, :512].reshape(512, KC, 128).transpose(2, 1, 0)
                ).astype(BF16NP),
                "wq8": wshuf8(Wq[:, s]),
                "wk8": wshuf8(Wk[:, s]),
                "wv8": wshuf8(Wv[:, s]),
                "wq16": wshuf16(Wq[:, s]),
                "wk16": wshuf16(Wk[:, s]),
                "wv16": wshuf16(Wv[:, s]),
                "wp": np.ascontiguousarray(
                    Wp[s, :].reshape(2, 128, D).transpose(1, 0, 2)
                ).astype(BF16NP),
                "cbias": np.ascontiguousarray(
                    WSC
                    * np.concatenate(
                        [bq[s].reshape(2, 128).T, bk[s].reshape(2, 128).T], 1
                    )
                ),
                "c128": c128v,
                "conesm": conesmv,
            }
        )

    return in_maps


def _gather(results, bv, Wp, bp):
    out = np.zeros((B, T, D), np.float32)
    for c in range(8):
        out[c // 4] += results[c]["y"]
    # softmax weights sum to exactly 1, so the V bias contributes the
    # constant row bv @ Wp — applied here instead of on-device
    const = np.asarray(bv, np.float32) @ np.asarray(Wp, np.float32)
    out += (const + np.asarray(bp, np.float32))[None, None, :]
    return out


def kernel(xp, Wq, bq, Wk, bk, Wv, bv, Wp, bp):
    nc = _get_nc()
    in_maps = _make_in_maps(xp, Wq, bq, Wk, bk, Wv, bv, Wp, bp)
    res = run_bass_kernel_spmd(nc, in_maps, list(range(8)))
    return _gather(res.results, bv, Wp, bp)


# revision 37
# speedup vs baseline: 1.0818x; 1.0490x over previous
"""Causal self-attention (B=2, T=2048, D_in=1152, D=1024, H=16) on 8 trn2 cores.

Sharding: 2-way data parallel over batch x 4-way tensor parallel over heads.
Core c handles batch b = c//4 and heads [4g, 4g+4) with g = c%4.

Per-core dataflow — hybrid precision so fp8 never touches short softmax rows:
  Query tile 0 (t < 512, small k_eff: errors don't average down) runs an
  all-bf16 pipeline; query tiles 1-3 (k_eff >= ~190) run fp8e4 with DoubleRow
  matmuls at 0.5 cyc/row. All weights are pre-scaled x32 on the host so fp8's
  e4m3 normal range covers their N(0,1/1152) entries; the resulting 1024x
  score scale folds into the exp scale, and onesm=1/32 folds the V prescale
  out during normalization. D_in is padded 1152->1280 = 5 DR chunk-pairs so
  fp8 projections never mix DR/non-DR in one psum accumulation group.
  QT = (32Wq)^T @ xp^T -> bf16 [128, 2, T] (head dims on partitions; cols
  0-511 from the bf16 proj, rest from fp8 DR proj); KT likewise. V stored as
  32v with a ones-column per head, in bf16 (blocks 0-3) and fp8 (all blocks).
  Scores transposed bf16: ST[k, q] = K Q^T per 128-row k-block; exp on ACT
  has bias -3 (cancels in softmax; keeps max exp ~e^4.6, under e4m3's 240
  max) and writes bf16 (qt0) or fp8 (qt>=1). Causal mask = one [128,2,256]
  mask-pair 0/1 multiply per diagonal group; block 2g+1's pre-diagonal
  stripe is zeroed so the paired DR att@V can't pick it up.
  OT~ = Vh~^T @ ex accumulates [65, 512] in PSUM — one DoubleRow matmul per
  two-block group for qt>=1 (the ex[:, jj, :] pair axis IS the DR layout),
  per-block bf16 matmuls for qt0; row 64 is the softmax row-sum. Normalize
  via reciprocal_approx_fast + rank-1 broadcast matmul, deferred into the
  filler stream. Y_partial = OT_all^T @ Wp in f32r as late filler.
Host sums the 4 partial Y per batch and adds bv@Wp + bp (exact: softmax
weights sum to 1, so the V bias contributes a constant row). Host
pre-shuffles all layouts so every DMA is 128 fully-contiguous lines.
"""

import functools as _ft
from collections import deque

import ml_dtypes
import numpy as np

import concourse.bass as bass
import concourse.mybir as mybir
import concourse.tile as tile
from concourse import bacc
from concourse.bass_utils import run_bass_kernel_spmd

F32 = mybir.dt.float32
F32R = mybir.dt.float32r
BF16 = mybir.dt.bfloat16
FP8 = mybir.dt.float8e4
AF = mybir.ActivationFunctionType
MUL = mybir.AluOpType.mult
DR = mybir.MatmulPerfMode.DoubleRow

B, T, DIN, D, H = 2, 2048, 1152, 1024, 16
HD = D // H           # 64 head dim
HLOC = 4              # heads per core
DLOC = HLOC * HD      # 256 local model dims
KC = DIN // 128       # 9 contraction chunks (bf16 path)
KCP = 10              # padded chunks (1280 = 5 DoubleRow pairs of 256)
NPAIR = KCP // 2
NT = T // 512         # 4 column tiles of 512
QC = T // 128         # 16 row chunks of 128
SCALE = 1.0 / np.sqrt(np.float32(HD))
WSC = 32.0            # host-side weight prescale for fp8 range
EBIAS = -3.0          # exp bias: cancels in softmax, keeps exp < e4m3 max

_CACHE = {}


def _build():
    nc = bacc.Bacc(None)

    xp8 = nc.dram_tensor("xp8", [128, 3, KCP, 512], FP8, kind="ExternalInput")
    xp16 = nc.dram_tensor("xp16", [128, KC, 512], BF16, kind="ExternalInput")
    wq8 = nc.dram_tensor("wq8", [128, KCP, DLOC], FP8, kind="ExternalInput")
    wk8 = nc.dram_tensor("wk8", [128, KCP, DLOC], FP8, kind="ExternalInput")
    wv8 = nc.dram_tensor("wv8", [128, KCP, DLOC], FP8, kind="ExternalInput")
    wq16 = nc.dram_tensor("wq16", [128, KC, DLOC], BF16, kind="ExternalInput")
    wk16 = nc.dram_tensor("wk16", [128, KC, DLOC], BF16, kind="ExternalInput")
    wv16 = nc.dram_tensor("wv16", [128, KC, DLOC], BF16, kind="ExternalInput")
    wp = nc.dram_tensor("wp", [128, 2, D], BF16, kind="ExternalInput")
    cbias = nc.dram_tensor("cbias", [128, 4], F32, kind="ExternalInput")
    c128 = nc.dram_tensor("c128", [128, 512], F32, kind="ExternalInput")
    conesm = nc.dram_tensor("conesm", [128, 256], BF16, kind="ExternalInput")
    y = nc.dram_tensor("y", [T, D], F32, kind="ExternalOutput")

    with tile.TileContext(nc) as tc:
        with (
            tc.tile_pool(name="const", bufs=1) as cpool,
            tc.tile_pool(name="work", bufs=2) as wpool,
            tc.tile_pool(name="exp", bufs=4) as epool,
            tc.tile_pool(name="stg", bufs=4) as spool,
            tc.tile_pool(name="psB", bufs=1, space="PSUM") as psB,
            tc.tile_pool(name="psC", bufs=2, space="PSUM") as psC,
            tc.tile_pool(name="psX", bufs=2, space="PSUM") as psX,
            nc.allow_low_precision(reason="fp8/bf16/f32r matmul pipeline"),
        ):
            t_wq8 = cpool.tile([128, KCP, DLOC], FP8, tag="t_wq8")
            t_wk8 = cpool.tile([128, KCP, DLOC], FP8, tag="t_wk8")
            t_wv8 = cpool.tile([128, KCP, DLOC], FP8, tag="t_wv8")
            t_wq16 = cpool.tile([128, KC, DLOC], BF16, tag="t_wq16")
            t_wk16 = cpool.tile([128, KC, DLOC], BF16, tag="t_wk16")
            t_wv16 = cpool.tile([128, KC, DLOC], BF16, tag="t_wv16")
            t_wp = cpool.tile([128, 2, D], BF16, tag="t_wp")
            t_cbias = cpool.tile([128, 4], F32, tag="t_cbias")
            t_c128 = cpool.tile([128, 512], F32R, tag="t_c128")
            t_conesm = cpool.tile([128, 256], BF16, tag="t_conesm")
            t_qt = cpool.tile([128, 2, T], BF16, tag="t_qt")
            t_kt = cpool.tile([128, 2, T], BF16, tag="t_kt")
            # HD+4: DoubleRow ldweights requires pair-axis stride %16 == 0
            # (HLOC*(HD+4) = 272 bytes); col 64 = softmax-sum ones, cols
            # 65-67 = dup ones (output rows 65-67 unused)
            t_v8 = cpool.tile([128, QC, HLOC, HD + 4], FP8, tag="t_v8")
            t_v16 = cpool.tile([128, 4, HLOC, HD + 1], BF16, tag="t_v16")
            t_ot = cpool.tile([128, 2, T], BF16, tag="t_ot")
            t_sums4 = cpool.tile([128, 512], F32, tag="t_sums4")
            t_recf = cpool.tile([128, 512], F32, tag="t_recf")
            t_rec4 = cpool.tile([128, 512], BF16, tag="t_rec4")

            t_bq = t_cbias[:, 0:2]
            t_bk = t_cbias[:, 2:4]
            t_onesm = t_conesm[:].rearrange("p (r c) -> p r c", r=4)
            t_tril = t_c128[:].rearrange("p (j c) -> p j c", j=2)

            # HAM warm-up: ~12 dummy matmuls on a memset tile run while the
            # first DMAs land, so the clock gate is at 8/8 when real matmuls
            # start (cold matmuls run at 1.2 GHz instead of 2.4)
            t_dmy = cpool.tile([128, 512], BF16, tag="t_dmy")
            nc.vector.memset(t_dmy[:], 0.5)
            p_dmy = psX.tile([128, 512], F32, tag="aux")
            for _ in range(10):
                nc.tensor.matmul(
                    p_dmy[:], t_dmy[:, 0:128], t_dmy[:], start=True, stop=True
                )

            # few LARGE DMAs: per-chunk loads are trigger-bound (~0.65us
            # per DMA_DIRECT2D issue + serialized ~1.3us/chunk transfers put
            # the last chunk ~18us out). Two halves per tensor match the
            # proj unit split while keeping trigger count low.
            t_xp0 = cpool.tile([128, KC, 512], BF16, tag="t_xp0")
            nc.sync.dma_start(t_wq16[:, 0 : KC // 2, :], wq16[:, 0 : KC // 2, :])
            nc.scalar.dma_start(t_xp0[:, 0 : KC // 2, :], xp16[:, 0 : KC // 2, :])
            nc.gpsimd.dma_start(t_wk16[:, 0 : KC // 2, :], wk16[:, 0 : KC // 2, :])
            nc.sync.dma_start(t_wq16[:, KC // 2 :, :], wq16[:, KC // 2 :, :])
            nc.scalar.dma_start(t_xp0[:, KC // 2 :, :], xp16[:, KC // 2 :, :])
            nc.gpsimd.dma_start(t_wk16[:, KC // 2 :, :], wk16[:, KC // 2 :, :])
            nc.sync.dma_start(t_wv16[:], wv16[:])
            nc.gpsimd.dma_start(t_cbias[:], cbias[:])
            nc.gpsimd.dma_start(t_c128[:], c128[:].bitcast(F32R))
            nc.gpsimd.dma_start(t_conesm[:], conesm[:])
            nc.gpsimd.dma_start(t_wq8[:], wq8[:])
            nc.gpsimd.dma_start(t_wk8[:], wk8[:])
            nc.gpsimd.dma_start(t_wv8[:], wv8[:])
            nc.gpsimd.memset(t_sums4[:], 1.0)
            # ones columns for every V block, written once through a staging
            # row (strided 1-byte memset is riskier than a strided copy)
            t_vones = cpool.tile([128, 256], F32, tag="t_vones")
            nc.gpsimd.memset(t_vones[:], 1.0)
            t_ebias = cpool.tile([128, 1], F32, tag="t_ebias")
            nc.gpsimd.memset(t_ebias[:], EBIAS)
            nc.vector.tensor_copy(
                out=t_v8[:, :, :, HD : HD + 4],
                in_=t_vones[:].rearrange("p (a b c) -> p a b c", a=QC, b=HLOC),
            )
            nc.vector.tensor_copy(
                out=t_v16[:, :, :, HD],
                in_=t_vones[:, 0:16].rearrange("p (a b) -> p a b", a=4),
            )

            def proj16():
                # bf16 projection for nt=0: accurate q/k cols 0-511 and
                # v blocks 0-3 (written to both the bf16 and fp8 V tiles)
                groups = []
                live = {}

                def qk_half(t_w, t_b, t_dst, m, half):
                    if half == 0:
                        p = psX.tile([128, 512], F32, tag="aux")
                        live[(id(t_w), m)] = p
                        ks = range(0, KC // 2)
                    else:
                        p = live.pop((id(t_w), m))
                        ks = range(KC // 2, KC)
                    for k in ks:
                        nc.tensor.matmul(
                            p[:],
                            t_w[:, k, 128 * m : 128 * m + 128],
                            t_xp0[:, k, :],
                            start=(k == 0),
                            stop=(k == KC - 1),
                        )
                    if half == 1:
                        # bias-add copy on DVE: the ACT queue is reserved for
                        # exp so proj epilogues never head-of-line block it
                        nc.vector.tensor_scalar(
                            t_dst[:, m, 0:512],
                            p[:],
                            t_b[:, m : m + 1],
                            None,
                            mybir.AluOpType.add,
                        )

                def v_half(tc4, half):
                    if half == 0:
                        p = psX.tile([128, 512], F32, tag="aux")
                        live[("v", tc4)] = p
                        ks = range(0, KC // 2)
                    else:
                        p = live.pop(("v", tc4))
                        ks = range(KC // 2, KC)
                    for k in ks:
                        nc.tensor.matmul(
                            p[:, :DLOC],
                            t_xp0[:, k, 128 * tc4 : 128 * tc4 + 128],
                            t_wv16[:, k, :],
                            start=(k == 0),
                            stop=(k == KC - 1),
                        )
                    if half == 1:
                        pv = p[:, :DLOC].rearrange("p (h d) -> p h d", h=HLOC)
                        nc.vector.tensor_copy(
                            out=t_v16[:, tc4, :, 0:HD], in_=pv
                        )
                        nc.vector.tensor_copy(
                            out=t_v8[:, tc4, :, 0:HD], in_=pv
                        )

                for t_w, t_b, t_dst in (
                    (t_wq16, t_bq, t_qt),
                    (t_wk16, t_bk, t_kt),
                ):
                    for m in range(2):
                        for half in range(2):
                            groups.append(
                                _ft.partial(qk_half, t_w, t_b, t_dst, m, half)
                            )
                for tc4 in range(4):
                    for half in range(2):
                        groups.append(_ft.partial(v_half, tc4, half))
                return groups

            def proj8(nt):
                # fp8 DoubleRow projection for nt>=1
                c0 = 512 * nt
                t_xp = wpool.tile([128, KCP, 512], FP8, tag="t_xp")
                nc.sync.dma_start(t_xp[:], xp8[:, nt - 1])
                groups = []
                live = {}

                def qk_half(t_w, t_b, t_dst, m, half, t_xp=t_xp, c0=c0):
                    if half == 0:
                        p = psX.tile([128, 512], F32, tag="aux")
                        live[(id(t_w), m)] = p
                        ps = range(0, 2)
                    else:
                        p = live.pop((id(t_w), m))
                        ps = range(2, NPAIR)
                    for pr in ps:
                        nc.tensor.matmul(
                            p[:],
                            t_w[:, 2 * pr : 2 * pr + 2, 128 * m : 128 * m + 128],
                            t_xp[:, 2 * pr : 2 * pr + 2, :],
                            start=(pr == 0),
                            stop=(pr == NPAIR - 1),
                            perf_mode=DR,
                        )
                    if half == 1:
                        nc.vector.tensor_scalar(
                            t_dst[:, m, c0 : c0 + 512],
                            p[:],
                            t_b[:, m : m + 1],
                            None,
                            mybir.AluOpType.add,
                        )

                def v_half(tc4, half, t_xp=t_xp, nt=nt):
                    tch = 4 * nt + tc4
                    if half == 0:
                        p = psX.tile([128, 512], F32, tag="aux")
                        live[("v", tc4)] = p
                        ps = range(0, 2)
                    else:
                        p = live.pop(("v", tc4))
                        ps = range(2, NPAIR)
                    for pr in ps:
                        nc.tensor.matmul(
                            p[:, :DLOC],
                            t_xp[
                                :, 2 * pr : 2 * pr + 2, 128 * tc4 : 128 * tc4 + 128
                            ],
                            t_wv8[:, 2 * pr : 2 * pr + 2, :],
                            start=(pr == 0),
                            stop=(pr == NPAIR - 1),
                            perf_mode=DR,
                        )
                    if half == 1:
                        nc.vector.tensor_copy(
                            out=t_v8[:, tch, :, 0:HD],
                            in_=p[:, :DLOC].rearrange("p (h d) -> p h d", h=HLOC),
                        )

                for t_w, t_b, t_dst in (
                    (t_wq8, t_bq, t_qt),
                    (t_wk8, t_bk, t_kt),
                ):
                    for m in range(2):
                        for half in range(2):
                            groups.append(
                                _ft.partial(qk_half, t_w, t_b, t_dst, m, half)
                            )
                for tc4 in range(4):
                    for half in range(2):
                        groups.append(_ft.partial(v_half, tc4, half))
                return groups

            pend = deque()       # proj groups: must drain on schedule
            pend_fqc = deque()   # output-proj chunks: reserved late filler

            # last-resort PE filler: a dependency-free dummy matmul keeps the
            # clock gate (HAM) at 8/8 — any PE gap over ~1us costs ~10us of
            # half-clock across every engine
            def dummy_mm():
                pd = psX.tile([128, 512], F32, tag="aux")
                nc.tensor.matmul(
                    pd[:], t_dmy[:, 0:128], t_dmy[:], start=True, stop=True
                )

            norm_emitted = set()

            def pop_filler(n, min_fqc=0, pad=False, fqc_ok=True):
                for _ in range(n):
                    if pend:
                        pend.popleft()()
                    elif (
                        fqc_ok
                        and len(pend_fqc) > min_fqc
                        # an O-proj unit is only valid once its query tile's
                        # normalization TTs have been emitted
                        and pend_fqc[0][0] in norm_emitted
                    ):
                        pend_fqc.popleft()[1]()
                    elif pad:
                        dummy_mm()

            # only what qt0-hf0 needs runs up front: Q-m0, K-m0 and V of
            # tile 0. The m=1 halves (first needed by qt0-hf1) and proj8(1..3)
            # become filler, so attention starts ~4us earlier.
            g16 = proj16()
            for i in (0, 1, 4, 5, *range(8, 16)):
                g16[i]()
            m1_left = [4]

            def _m1_unit(fn):
                def run():
                    fn()
                    m1_left[0] -= 1
                return run

            pend.extend(_m1_unit(g16[i]) for i in (2, 3, 6, 7))
            pend.extend(proj8(1))
            # wp issued here: lands during qt0, mostly off the proj window
            nc.gpsimd.dma_start(t_wp[:], wp[:])

            FLAGS = {"flush": False}

            def final_half(qc, n2, ty):
                # self-contained 512-col half of the output projection:
                # alloc->accumulate->stage within one filler unit so the aux
                # pool never rotates onto a live accumulation. N=512 keeps
                # the ldweights pipelined (a 256-col split costs ~2x).
                py = psX.tile([128, 512], F32, tag="aux")
                for c in range(2):
                    nc.tensor.matmul(
                        py[:],
                        t_ot[:, c, 128 * qc : 128 * qc + 128],
                        t_wp[:, c, 512 * n2 : 512 * n2 + 512],
                        start=(c == 0),
                        stop=(c == 1),
                    )
                # at flush alternate DVE/ACT so neither engine gates the drain
                if FLAGS["flush"] and n2 % 2 == 0:
                    nc.scalar.copy(ty[:, 512 * n2 : 512 * n2 + 512], py[:])
                else:
                    nc.vector.tensor_copy(
                        out=ty[:, 512 * n2 : 512 * n2 + 512], in_=py[:]
                    )
                if n2 == 1:
                    # sync queue is idle mid-run; gpsimd also runs the
                    # critical-path tril multiplies
                    nc.sync.dma_start(y[128 * qc : 128 * qc + 128, :], ty[:])

            def final_qc(qc, qt):
                # two poppable halves so late filler stays granular
                ty = wpool.tile([128, D], F32, tag="ty")
                for n2 in range(2):
                    pend_fqc.append(
                        (qt, _ft.partial(final_half, qc, n2, ty))
                    )

            def norm_qt(qt, stages):
                # deferred a full iteration, so 1/rowsum is long since ready
                q0 = 512 * qt
                for r in range(4):
                    hfr, pp = divmod(r, 2)
                    hp = 64 * pp
                    bcp = psX.tile([128, 512], F32, tag="aux")
                    nc.tensor.matmul(
                        bcp[0:64, :],
                        t_onesm[32 * r : 32 * r + 32, r, :],
                        t_rec4[32 * r : 32 * r + 32, :],
                        start=True,
                        stop=True,
                        tile_position=(32 * r, 0),
                    )
                    nc.vector.tensor_tensor(
                        t_ot[hp : hp + 64, hfr, q0 : q0 + 512],
                        bcp[0:64, :],
                        stages[r][:],
                        MUL,
                    )
                    if r == 1:
                        pop_filler(1)
                norm_emitted.add(qt)

            POPS = (4, 2, 2, 1)
            FQC_MIN = (0, 0, 0, 2)

            def make_stg(qt, hf, exs):
                q0 = 512 * qt
                fp8path = qt > 0

                def emit_stg(pp, g):
                    hp = 64 * pp
                    st = psB.tile([128, 2, 512], F32, tag=f"st{pp}")
                    # diagonal pairs skip columns below the pair's live
                    # region; block 2g+1's dead 128-col stripe above that
                    # is zeroed by the mask multiply (it must be exp'd
                    # anyway: the paired DR att@V reads both rows)
                    o = 256 * (g - 2 * qt) if g >= 2 * qt else 0
                    for jj in range(2):
                        j = 2 * g + jj
                        nc.tensor.matmul(
                            st[:, jj, o:],
                            t_kt[hp : hp + 64, hf, 128 * j : 128 * j + 128],
                            t_qt[hp : hp + 64, hf, q0 + o : q0 + 512],
                            start=True,
                            stop=True,
                        )
                    ex = epool.tile(
                        [128, 2, 512],
                        FP8 if fp8path else BF16,
                        tag=f"ex{pp}_{int(fp8path)}",
                    )
                    if g >= 2 * qt:
                        nc.scalar.activation(
                            ex[:, :, o:], st[:, :, o:], AF.Exp,
                            scale=float(SCALE / (WSC * WSC)),
                            bias=t_ebias[:],
                        )
                        eng = nc.vector if (pp == 0) else nc.gpsimd
                        eng.tensor_tensor(
                            ex[:, :, o : o + 256],
                            ex[:, :, o : o + 256],
                            t_tril,
                            MUL,
                        )
                    else:
                        nc.scalar.activation(
                            ex[:], st[:], AF.Exp,
                            scale=float(SCALE / (WSC * WSC)),
                            bias=t_ebias[:],
                        )
                    exs[(pp, g)] = ex

                return emit_stg

            sections = [(qt, hf) for qt in range(NT) for hf in range(2)]
            last_norm = None
            carry = None      # next section's pre-emitted exs dict
            stages = {}
            for si, (qt, hf) in enumerate(sections):
                # the pair's two heads run as independent, interleaved
                # ST->exp->OT chains: while one head's exp is on ACT, the
                # PE works the sibling head, so neither engine stalls.
                ngrp = 2 * qt + 2
                q0 = 512 * qt
                fp8path = qt > 0
                if hf == 0:
                    stages = {}
                    if qt + 2 < NT:
                        pend.extend(proj8(qt + 2))
                exs = carry if carry is not None else {}
                carry = None
                emit_stg = make_stg(qt, hf, exs)
                ots = {}

                def emit_otg(pp, g, first, last, qt=qt, hf=hf,
                             fp8path=fp8path, exs=exs, ots=ots):
                    h = 2 * hf + pp
                    ex = exs.pop((pp, g))
                    if fp8path:
                        o = 256 if g == 2 * qt + 1 else 0
                        nc.tensor.matmul(
                            ots[pp][:, o:],
                            t_v8[:, 2 * g : 2 * g + 2, h, :],
                            ex[:, :, o:],
                            start=first,
                            stop=last,
                            perf_mode=DR,
                        )
                    else:
                        for jj in range(2):
                            j = 2 * g + jj
                            off = 128 * j
                            nc.tensor.matmul(
                                ots[pp][0:65, off:],
                                t_v16[:, j, h, :],
                                ex[:, jj, off:],
                                start=(first and jj == 0),
                                stop=(last and jj == 1),
                            )

                if si == 0:
                    # first section: no previous section pre-emitted for us
                    emit_stg(0, 0)
                    emit_stg(1, 0)
                pop_filler(1, FQC_MIN[qt])
                if qt == 0 and hf == 1:
                    # hf1 scores read t_qt/t_kt m=1: those projection
                    # units must be emitted (not just queued) first
                    while m1_left[0] > 0:
                        pop_filler(1, fqc_ok=False)
                if hf == 0 and last_norm is not None:
                    # the qt-1 norm runs here: this section's first scores
                    # were pre-emitted by the previous section, so the PE has
                    # ready work while the rec4 DVE chain completes
                    last_norm()
                    last_norm = None
                for pp in range(2):
                    ots[pp] = psC.tile(
                        [68, 512], F32, tag="ot", name=f"ot_{qt}_{hf}_{pp}"
                    )
                for g in range(1, ngrp):
                    for pp in range(2):
                        emit_stg(pp, g)
                    for pp in range(2):
                        emit_otg(pp, g - 1, g == 1, False)
                    pop_filler(POPS[qt], FQC_MIN[qt], pad=True)
                pop_filler(2, FQC_MIN[qt], pad=True)
                for pp in range(2):
                    emit_otg(pp, ngrp - 1, ngrp == 1, True)
                # pre-emit the NEXT section's first score group before this
                # section's epilogue: the ACT queue stays fed across the
                # hf/qt boundary instead of idling ~1-2us
                if si + 1 < len(sections):
                    nqt, nhf = sections[si + 1]
                    if nqt == 0 and nhf == 1:
                        # the pre-emitted hf1 scores read t_qt/t_kt m=1:
                        # those projection units must be emitted first
                        while m1_left[0] > 0:
                            pop_filler(1, fqc_ok=False)
                    carry = {}
                    nstg = make_stg(nqt, nhf, carry)
                    nstg(0, 0)
                    nstg(1, 0)
                for pp in range(2):
                    h = 2 * hf + pp
                    # denominator row straight off PSUM so the reciprocal
                    # chain never waits on the big staging copies
                    nc.vector.tensor_copy(
                        out=t_sums4[32 * h : 32 * h + 1, :],
                        in_=ots[pp][64:65, :],
                    )
                    stage = spool.tile([64, 512], F32, tag="stg")
                    nc.vector.tensor_copy(
                        out=stage[:], in_=ots[pp][0:64, :]
                    )
                    stages[h] = stage
                if hf == 1:
                    # full-width fast reciprocal (custom-DVE ops misbehave
                    # on offset partition slices; per-lane cost is equal),
                    # rounded to f32r for the broadcast matmul
                    nc.vector.reciprocal_approx_fast(
                        out=t_recf[:], in_=t_sums4[:]
                    )
                    nc.vector.tensor_copy(out=t_rec4[:], in_=t_recf[:])
                    last_norm = _ft.partial(norm_qt, qt, stages)
                    for qc in range(4 * qt, 4 * qt + 4):
                        final_qc(qc, qt)

            pop_filler(4)
            last_norm()
            for _ in range(4):
                dummy_mm()
            FLAGS["flush"] = True
            while pend or pend_fqc:
                pop_filler(1)

    nc.compile()
    return nc


def _get_nc():
    if "nc" not in _CACHE:
        _CACHE["nc"] = _build()
    return _CACHE["nc"]


def _make_in_maps(xp, Wq, bq, Wk, bk, Wv, bv, Wp, bp):
    xp = np.asarray(xp, np.float32)
    Wq, Wk, Wv, Wp = (np.asarray(a, np.float32) for a in (Wq, Wk, Wv, Wp))
    bq, bk, bv, bp = (np.asarray(a, np.float32) for a in (bq, bk, bv, bp))
    FP8NP = ml_dtypes.float8_e4m3
    BF16NP = ml_dtypes.bfloat16

    tril = (np.arange(128)[None, :] >= np.arange(128)[:, None]).astype(
        np.float32
    )
    # mask-pair for a diagonal two-block group at column offset o:
    # block 2g (jj=0): cols 0:128 tril, 128:256 keep
    # block 2g+1 (jj=1): cols 0:128 dead (the DR pair reads them), 128:256 tril
    trilpair = np.empty((128, 2, 256), np.float32)
    trilpair[:, 0, 0:128] = tril
    trilpair[:, 0, 128:256] = 1.0
    trilpair[:, 1, 0:128] = 0.0
    trilpair[:, 1, 128:256] = tril
    # onesm = 1/WSC: folds the x32 V prescale out during normalization
    onesmv = np.zeros((128, 4, 64), np.float32)
    for r in range(4):
        onesmv[32 * r, r, :] = 1.0 / WSC
    c128v = trilpair.reshape(128, 512)
    conesmv = onesmv.reshape(128, 256).astype(BF16NP)

    def wshuf8(Wc):
        wpad = np.concatenate(
            [WSC * Wc, np.zeros((KCP * 128 - DIN, DLOC), np.float32)], axis=0
        )
        return np.ascontiguousarray(
            wpad.reshape(KCP, 128, DLOC).transpose(1, 0, 2)
        ).astype(FP8NP)

    def wshuf16(Wc):
        return np.ascontiguousarray(
            (WSC * Wc).reshape(KC, 128, DLOC).transpose(1, 0, 2)
        ).astype(BF16NP)

    in_maps = []
    for c in range(8):
        b, g = divmod(c, 4)
        s = slice(DLOC * g, DLOC * (g + 1))
        xpad = np.concatenate(
            [xp[b], np.zeros((T, KCP * 128 - DIN), np.float32)], axis=1
        )
        in_maps.append(
            {
                "xp8": np.ascontiguousarray(
                    xpad[512:].reshape(3, 512, KCP, 128).transpose(3, 0, 2, 1)
                ).astype(FP8NP),
                "xp16": np.ascontiguousarray(
                    xp[b, :512].reshape(512, KC, 128).transpose(2, 1, 0)
                ).astype(BF16NP),
                "wq8": wshuf8(Wq[:, s]),
                "wk8": wshuf8(Wk[:, s]),
                "wv8": wshuf8(Wv[:, s]),
                "wq16": wshuf16(Wq[:, s]),
                "wk16": wshuf16(Wk[:, s]),
                "wv16": wshuf16(Wv[:, s]),
                "wp": np.ascontiguousarray(
                    Wp[s, :].reshape(2, 128, D).transpose(1, 0, 2)
                ).astype(BF16NP),
                "cbias": np.ascontiguousarray(
                    WSC
                    * np.concatenate(
                        [bq[s].reshape(2, 128).T, bk[s].reshape(2, 128).T], 1
                    )
                ),
                "c128": c128v,
                "conesm": conesmv,
            }
        )

    return in_maps


def _gather(results, bv, Wp, bp):
    out = np.zeros((B, T, D), np.float32)
    for c in range(8):
        out[c // 4] += results[c]["y"]
    # softmax weights sum to exactly 1, so the V bias contributes the
    # constant row bv @ Wp — applied here instead of on-device
    const = np.asarray(bv, np.float32) @ np.asarray(Wp, np.float32)
    out += (const + np.asarray(bp, np.float32))[None, None, :]
    return out


def kernel(xp, Wq, bq, Wk, bk, Wv, bv, Wp, bp):
    nc = _get_nc()
    in_maps = _make_in_maps(xp, Wq, bq, Wk, bk, Wv, bv, Wp, bp)
    res = run_bass_kernel_spmd(nc, in_maps, list(range(8)))
    return _gather(res.results, bv, Wp, bp)
